# revision 27
# baseline (speedup 1.0000x reference)
"""Bass/Trainium2 kernel for single-token (decode) self-attention with a
large KV cache, RoPE, and output projection.

Sharding: tensor-parallel over heads. 16 heads / 8 cores = 2 heads per
core; every core sees all 8 batch rows. Per-core HBM traffic is dominated
by its KV-cache slice, which the host downcasts to bf16 (2 x ~16.8MB per
core) — the rel-err budget (2e-2) comfortably absorbs the ~1% noise and
the DMA floor halves vs fp32. QKV weights are sliced by head rows, Wo by
columns (row-parallel out projection); each core returns a partial
(8, 1024) output and the host sums the 8 partials.

Kernel structure per core:
  - q/k/v = x @ W.T + b via PE (fp32), RoPE on DVE with host cos/sin rows
    (q rows also carry the 1/sqrt(hd) attention scale); payload rows are
    broadcast to 128 partitions via one-hot PE matmuls and cast to bf16.
  - K arrives as [128, 64, 64] bf16 with key j = 64*partition + col; V as
    [128, 64, 65] with a host-appended ones column so the softmax
    denominator accumulates in PSUM alongside the context for free.
  - scores: one bf16 tensor_tensor multiply against a 0-stride broadcast
    view of q (DVE 2x perf mode), then a 6-level add-tree over hd
    (levels 1-3 bf16/2x, levels 4-6 fp32 out for precision) — avoids
    tensor_reduce, which is capped at 1x mode.
  - softmax without max subtraction (scores are O(1) by construction);
    exp on ACT producing bf16 probabilities.
  - attn @ V: one PE matmul per 128-key slab, lhsT = prob column
    (LDWEIGHTS overlaps on the second SBUF read port), rhs = [V | 1]
    slab, accumulated into a (1, 16*65) PSUM row holding all 16
    (batch, head) contexts and denominators; the new token contributes
    one extra [1,1]x[1,65] matmul per pair.
  - normalize by 1/denominator, PE-transpose the context row, out-
    projection partial via PE against the host-transposed Wo slice.
"""

import functools
import os
import sys

import numpy as np

for _p in ("/opt/trn_rl_repo", "/root/.axon_site/_ro/trn_rl_repo"):
    if os.path.isdir(_p) and _p not in sys.path:
        sys.path.insert(0, _p)

from contextlib import ExitStack

import concourse.tile as tile
from concourse import bacc, mybir
from concourse.bass_utils import run_bass_kernel_spmd

B, S, D, H, PAST = 8, 1, 1024, 16, 8192
HD = 64
NCORES = 8
HPC = H // NCORES          # heads per core = 2
LP = HPC * HD              # local projection width = 128
NPAIR = B * HPC            # 16 (batch, local-head) problems per core
NCOL = PAST // 128         # 64 keys per partition = score columns per pair
VW = HD + 1                # V slab width incl. denominator ones column
# payload per batch: [q(128) | k(128) | v0,1(65) pad | v1,1(65) pad | e0 _ e1 _]
QBW = 392
PV0 = 2 * LP               # 256: v-block of head 0 (65 wide, ones at +64)
PV1 = 2 * LP + 66          # 322: v-block of head 1
PE0 = 388                  # exp(s_new) head 0; head 1 at 390

F32 = mybir.dt.float32
BF16 = mybir.dt.bfloat16
MULT = mybir.AluOpType.mult
ADD = mybir.AluOpType.add
EXP = mybir.ActivationFunctionType.Exp
ACOPY = mybir.ActivationFunctionType.Copy


def _normalize_pair(nc, ctx_ps, ctxn, dinv, ctxT_ps, ident, p):
    """ctxn[64p:64p+64] = ctx_ps pair block / its denominator; transpose
    the batch row once both of its head pairs are normalized."""
    nc.vector.reciprocal(
        dinv[0:1, p : p + 1], ctx_ps[0:1, VW * p + HD : VW * (p + 1)]
    )
    nc.vector.tensor_scalar_mul(
        ctxn[0:1, HD * p : HD * (p + 1)],
        ctx_ps[0:1, VW * p : VW * p + HD],
        dinv[0:1, p : p + 1],
    )
    if p % HPC == HPC - 1:
        b = p // HPC
        nc.tensor.transpose(
            ctxT_ps[:, b : b + 1],
            ctxn[0:1, 128 * b : 128 * (b + 1)],
            ident[0:1, 0:1],
        )


def _build_bass():
    nc = bacc.Bacc(
        "TRN2", target_bir_lowering=False, debug=False, num_devices=NCORES
    )

    # weights/x arrive partition-major so every DMA line is contiguous
    d_wq = nc.dram_tensor("wq", (128, 8, LP), BF16, kind="ExternalInput").ap()
    d_wk = nc.dram_tensor("wk", (128, 8, LP), BF16, kind="ExternalInput").ap()
    d_wv = nc.dram_tensor("wv", (128, 8, LP), BF16, kind="ExternalInput").ap()
    d_wo = nc.dram_tensor("wo", (128, 8, 128), BF16, kind="ExternalInput").ap()
    d_xt = nc.dram_tensor("xt", (128, 8, B), BF16, kind="ExternalInput").ap()
    # c8: [rope(512) | bqkv(384) | eall(1024)] ; c128: [ident | ones]
    d_c8 = nc.dram_tensor("c8", (B, 1920), F32, kind="ExternalInput").ap()
    d_c128 = nc.dram_tensor("c128", (128, 129), F32, kind="ExternalInput").ap()
    d_pk = nc.dram_tensor(
        "pk", (B, HPC, 128, NCOL, HD), BF16, kind="ExternalInput"
    ).ap()
    d_pv = nc.dram_tensor(
        "pv", (B, HPC, 128, NCOL, VW), BF16, kind="ExternalInput"
    ).ap()
    d_out = nc.dram_tensor("out", (B, D), F32, kind="ExternalOutput").ap()

    with tile.TileContext(nc) as tc:
        with ExitStack() as ctx:
            const = ctx.enter_context(tc.tile_pool(name="const", bufs=1))
            small = ctx.enter_context(tc.tile_pool(name="small", bufs=1))
            wt = ctx.enter_context(tc.tile_pool(name="wt", bufs=1))
            kpool = ctx.enter_context(tc.tile_pool(name="kpool", bufs=6))
            vpool = ctx.enter_context(tc.tile_pool(name="vpool", bufs=7))
            prpool = ctx.enter_context(tc.tile_pool(name="prpool", bufs=2))
            t1pool = ctx.enter_context(tc.tile_pool(name="t1pool", bufs=2))
            t2pool = ctx.enter_context(tc.tile_pool(name="t2pool", bufs=2))
            t3pool = ctx.enter_context(tc.tile_pool(name="t3pool", bufs=3))
            scpool = ctx.enter_context(tc.tile_pool(name="scpool", bufs=3))
            atpool = ctx.enter_context(tc.tile_pool(name="atpool", bufs=4))

            # ---- constants (DMA queue order: small/critical first) --------
            c128 = const.tile([128, 129], F32)
            nc.sync.dma_start(c128[:], d_c128[:])
            c8 = const.tile([B, 1920], F32)
            nc.sync.dma_start(c8[:], d_c8[:])
            ident = c128[:, 0:128]
            rope = c8[:, 0 : 4 * LP]
            bias = c8[:, 4 * LP : 7 * LP]
            eall = c8[:, 7 * LP : 7 * LP + B * 128]

            # ---- prologue: projections, RoPE, bcast --------------------
            with ExitStack() as pctx:
                ps_p = pctx.enter_context(
                    tc.tile_pool(name="ps_p", bufs=1, space="PSUM")
                )
                ps_bc = pctx.enter_context(
                    tc.tile_pool(name="ps_bc", bufs=2, space="PSUM")
                )

                xt = small.tile([128, 8, B], BF16)
                nc.sync.dma_start(xt[:], d_xt[:])
                # Host supplies weights already transposed (in-dim on
                # partitions): wq[p, j, i] = Wq_c[i, 128j+p].
                wts = {}
                for nm, dram in (("q", d_wq), ("k", d_wk), ("v", d_wv)):
                    wtr = wt.tile([128, 8, LP], BF16, tag=f"wt_{nm}")
                    nc.sync.dma_start(wtr[:], dram[:])
                    wts[nm] = wtr

                # payload pads/ones first: no deps, runs during DMA wait
                payload = small.tile([B, QBW], F32)
                nc.vector.memset(payload[:, PV0 + HD : PV0 + HD + 1], 1.0)
                nc.vector.memset(payload[:, PV1 + HD : PV1 + HD + 1], 1.0)
                nc.vector.memset(payload[:, PV1 - 1 : PV1], 0.0)
                nc.vector.memset(payload[:, PV1 + HD + 1 : PE0], 0.0)
                nc.vector.memset(payload[:, PE0 : QBW], 0.0)

                # qkv projection: out (8, 384) = x @ [Wq|Wk|Wv].T
                qkv_ps = ps_p.tile([B, 3 * LP], F32, tag="qkv_ps")
                for i, nm in enumerate(("q", "k", "v")):
                    for j in range(8):
                        nc.tensor.matmul(
                            qkv_ps[:, LP * i : LP * (i + 1)],
                            xt[:, j, :],
                            wts[nm][:, j, :],
                            start=(j == 0),
                            stop=(j == 7),
                        )
                qkv = small.tile([B, 3 * LP], F32)
                nc.vector.tensor_tensor(qkv[:], qkv_ps[:], bias[:], ADD)

                # RoPE on q and k; payload = [rot(q) | rot(k) | v+ones | exp]
                swp = small.tile([B, 2 * LP], F32)  # [q | k] halves swapped
                for i in range(2):  # q, k
                    src = qkv[:, LP * i : LP * (i + 1)].rearrange(
                        "p (h t f) -> p h t f", h=HPC, t=2
                    )
                    dst = swp[:, LP * i : LP * (i + 1)].rearrange(
                        "p (h t f) -> p h t f", h=HPC, t=2
                    )
                    nc.vector.tensor_copy(dst[:, :, 0, :], src[:, :, 1, :])
                    nc.vector.tensor_copy(dst[:, :, 1, :], src[:, :, 0, :])
                tmp = small.tile([B, 2 * LP], F32)
                # tmp = swapped * S ; payload[0:256] = qk * C + tmp
                nc.vector.tensor_tensor(
                    tmp[:], swp[:], rope[:, 2 * LP : 4 * LP], MULT
                )
                nc.vector.tensor_tensor(
                    payload[:, 0 : 2 * LP],
                    qkv[:, 0 : 2 * LP],
                    rope[:, 0 : 2 * LP],
                    MULT,
                )
                nc.vector.tensor_tensor(
                    payload[:, 0 : 2 * LP],
                    payload[:, 0 : 2 * LP],
                    tmp[:],
                    ADD,
                )
                # v blocks (ones columns + pads preset above)
                nc.vector.tensor_copy(
                    payload[:, PV0 : PV0 + HD], qkv[:, 2 * LP : 2 * LP + HD]
                )
                nc.vector.tensor_copy(
                    payload[:, PV1 : PV1 + HD],
                    qkv[:, 2 * LP + HD : 2 * LP + 2 * HD],
                )

                # new-token scores s_new = 0.125 * rot(q).rot(k) per head
                snew = small.tile([B, HPC], F32)
                stt = small.tile([B, HD], F32)
                # q in payload is pre-scaled by 0.125 (folded into rope C/S)
                for hp in range(HPC):
                    nc.vector.scalar_tensor_tensor(
                        out=stt[:],
                        in0=payload[:, LP + HD * hp : LP + HD * (hp + 1)],
                        scalar=1.0,
                        in1=payload[:, HD * hp : HD * (hp + 1)],
                        op0=MULT,
                        op1=MULT,
                        accum_out=snew[:, hp : hp + 1],
                    )
                nc.scalar.activation(
                    payload[:, PE0 : PE0 + 4].rearrange(
                        "p (h t) -> p h t", t=2
                    )[:, :, 0:1],
                    snew[:].rearrange("p (h t) -> p h t", t=1),
                    EXP,
                )

                # broadcast payload rows to all 128 partitions, cast to bf16
                # (per-batch tiles so pair 0 starts before batch 7 lands;
                # casts on ACT so the DVE is free for the first score mult)
                qbs = []
                for b in range(B):
                    bc = ps_bc.tile([128, QBW], F32, tag="bc")
                    nc.tensor.matmul(
                        bc[:],
                        eall[:, 128 * b : 128 * (b + 1)],
                        payload[:],
                        start=True,
                        stop=True,
                    )
                    qb = const.tile([128, QBW], BF16, tag=f"qb{b}")
                    nc.scalar.activation(qb[:], bc[:], ACOPY)
                    qbs.append(qb)

            # ---- main attention loop over the 16 (batch, head) pairs ------
            # ctx_ps row: cols [65p, 65p+64) = context of pair p,
            #             col 65p+64         = softmax denominator of pair p
            ps_ctx = ctx.enter_context(
                tc.tile_pool(name="ps_ctx", bufs=1, space="PSUM")
            )
            ctx_ps = ps_ctx.tile([1, NPAIR * VW], F32)
            ctxT_ps = ps_ctx.tile([128, B], F32, tag="ctxT_ps")
            dinv = small.tile([1, NPAIR], F32)
            ctxn = small.tile([1, NPAIR * HD], F32)

            # Wo tile loads late (issued mid-loop): only the epilogue needs it
            wot = wt.tile([128, 8, 128], BF16, tag="wt_o")

            for p in range(NPAIR):
                b, hp = divmod(p, HPC)
                qb = qbs[b]

                kt = kpool.tile([128, NCOL, HD], BF16, tag="kt")
                nc.sync.dma_start(kt[:], d_pk[b, hp])
                vt = vpool.tile([128, NCOL, VW], BF16, tag="vt")
                nc.sync.dma_start(vt[:], d_pv[b, hp])
                if p == NPAIR - 3:
                    nc.sync.dma_start(wot[:], d_wo[:])

                qslice = qb[:, HD * hp : HD * (hp + 1)]
                qbc = qslice.rearrange("p (o d) -> p o d", o=1).broadcast_to(
                    [128, NCOL, HD]
                )

                # scores: bf16 multiply (2x mode) + 6-level add-tree over hd
                prod = prpool.tile([128, NCOL, HD], BF16, tag="prod")
                nc.vector.tensor_tensor(prod[:], kt[:], qbc, MULT)
                t1 = t1pool.tile([128, NCOL, 32], BF16, tag="t1")
                nc.vector.tensor_tensor(
                    t1[:], prod[:, :, 0:32], prod[:, :, 32:64], ADD
                )
                # HAM keep-alive: trivial matmuls dependent on mid-pair DVE
                # tiles, so the PE executes them *inside* its inter-chain
                # idle window and the 4096-cycle activity monitor never
                # re-throttles the clock to 1.2 GHz
                ka = ps_ctx.tile([1, 2], F32, tag="keepalive")
                nc.tensor.matmul(
                    ka[:], t1[0:1, 0, 0:1], t1[0:1, 0, 0:2],
                    start=True, stop=True,
                )
                t2 = t2pool.tile([128, NCOL, 16], BF16, tag="t2")
                nc.vector.tensor_tensor(
                    t2[:], t1[:, :, 0:16], t1[:, :, 16:32], ADD
                )
                t3 = t3pool.tile([128, NCOL, 8], BF16, tag="t3")
                nc.vector.tensor_tensor(
                    t3[:], t2[:, :, 0:8], t2[:, :, 8:16], ADD
                )
                t4 = t1pool.tile([128, NCOL, 4], BF16, tag="t4")
                nc.vector.tensor_tensor(
                    t4[:], t3[:, :, 0:4], t3[:, :, 4:8], ADD
                )
                t5 = t2pool.tile([128, NCOL, 2], BF16, tag="t5")
                nc.vector.tensor_tensor(
                    t5[:], t4[:, :, 0:2], t4[:, :, 2:4], ADD
                )
                sc = scpool.tile([128, NCOL], F32, tag="sc")
                scv = sc[:].rearrange("p (j o) -> p j o", o=1)
                nc.vector.tensor_tensor(
                    scv, t5[:, :, 0:1], t5[:, :, 1:2], ADD
                )
                nc.tensor.matmul(
                    ka[:], sc[0:1, 0:1], sc[0:1, 0:2], start=True, stop=True
                )

                at = atpool.tile([128, NCOL], BF16, tag="at")
                nc.scalar.activation(at[:], sc[:], EXP)

                # attn @ V (+ denominator via the ones column)
                cslice = ctx_ps[0:1, VW * p : VW * (p + 1)]
                for j in range(NCOL):
                    nc.tensor.matmul(
                        cslice,
                        at[:, j : j + 1],
                        vt[:, j, :],
                        start=(j == 0),
                        stop=False,
                    )
                # new token: exp(s_new) * [v_new | 1]
                pv = PV0 if hp == 0 else PV1
                nc.tensor.matmul(
                    cslice,
                    qb[0:1, PE0 + 2 * hp : PE0 + 2 * hp + 1],
                    qb[0:1, pv : pv + VW],
                    start=False,
                    stop=True,
                )

                # normalize pair p-3 (delayed so the DVE never stalls on the
                # in-flight PE accumulation chain of recent pairs)
                if p >= 3:
                    _normalize_pair(nc, ctx_ps, ctxn, dinv, ctxT_ps, ident, p - 3)

            for pp in range(NPAIR - 3, NPAIR):
                _normalize_pair(nc, ctx_ps, ctxn, dinv, ctxT_ps, ident, pp)

            # ---- finalize: out-projection --------------------------------
            ctxT = small.tile([128, B], BF16)
            nc.vector.tensor_copy(ctxT[:], ctxT_ps[:])

            outsb = small.tile([B, D], F32)
            for half in range(2):
                op_ps = ps_ctx.tile([B, 512], F32, tag="op_ps")
                nc.tensor.matmul(
                    op_ps[:],
                    ctxT[:],
                    wot[:, 4 * half : 4 * (half + 1), :],
                    start=True,
                    stop=True,
                )
                nc.vector.tensor_copy(
                    outsb[:, 512 * half : 512 * (half + 1)], op_ps[:]
                )
            nc.sync.dma_start(d_out[:], outsb[:])

    nc.compile()
    return nc


@functools.lru_cache(maxsize=1)
def _get_nc():
    return _build_bass()


def _rope_tables():
    """cos/sin rows for position PAST, mirroring reference.py's fp32 jax
    arithmetic so the tables round identically."""
    import jax
    import jax.numpy as jnp

    pos = (PAST + jnp.arange(S)).astype(jnp.float32)
    inv_freq = 1.0 / (
        10000.0 ** (jnp.arange(0, HD, 2, dtype=jnp.float32) / HD)
    )
    ang = pos[:, None] * inv_freq[None, :]
    cos32 = np.asarray(jnp.cos(ang))[0]
    sin32 = np.asarray(jnp.sin(ang))[0]
    cos64 = np.concatenate([cos32, cos32])
    ssin64 = np.concatenate([-sin32, sin32])
    return cos64.astype(np.float32), ssin64.astype(np.float32)


def _install_ntff_hook_shim():
    """The agent image's antenv stub lacks axon_hooks, which degrades
    run_bass_kernel_spmd(trace=True) into an ImportError. Provide the
    module and register the ctypes-based NTFF hook from trn_agent_boot."""
    import types

    try:
        import antenv.axon_hooks  # noqa: F401

        return
    except ImportError:
        pass
    try:
        import antenv
        from trn_agent_boot.trn_boot import _ntff_profile_via_ctypes

        mod = types.ModuleType("antenv.axon_hooks")
        _state = {"hook": _ntff_profile_via_ctypes("/opt/axon/libaxon_pjrt.so")}
        mod.get_axon_ntff_profile_hook = lambda: _state["hook"]
        mod.set_axon_ntff_profile_hook = lambda h: _state.update(hook=h)
        sys.modules["antenv.axon_hooks"] = mod
        antenv.axon_hooks = mod
    except Exception as e:  # profiling is best-effort
        print(f"ntff hook shim failed: {e}", file=sys.stderr)


@functools.lru_cache(maxsize=1)
def _bf16():
    import ml_dtypes

    return np.dtype(ml_dtypes.bfloat16)


def kernel(x, Wq, bq, Wk, bk, Wv, bv, Wo, bo, past_k, past_v):
    x = np.asarray(x, np.float32).reshape(B, D)
    Wq = np.asarray(Wq, np.float32)
    Wk = np.asarray(Wk, np.float32)
    Wv = np.asarray(Wv, np.float32)
    Wo = np.asarray(Wo, np.float32)
    bq = np.asarray(bq, np.float32)
    bk = np.asarray(bk, np.float32)
    bv = np.asarray(bv, np.float32)
    bo = np.asarray(bo, np.float32)

    bf16 = _bf16()
    # K: (B, H, 128, 64, 64) bf16 with key = 64*partition + col
    pk_all = np.ascontiguousarray(past_k, np.float32).reshape(
        B, H, 128, NCOL, HD
    ).astype(bf16)
    # V: (B, H, 128, 64, 65) bf16 with a ones column appended per slab
    pv_all = np.empty((B, H, 128, NCOL, VW), dtype=bf16)
    pv_all[..., :HD] = np.ascontiguousarray(past_v, np.float32).reshape(
        B, H, 128, NCOL, HD
    ).astype(bf16)
    pv_all[..., HD] = np.float32(1.0)

    cos64, ssin64 = _rope_tables()
    # C/S for the q columns carry the 1/sqrt(hd) attention scale
    cq = np.tile(cos64, HPC) * np.float32(0.125)
    ck = np.tile(cos64, HPC)
    sq = np.tile(ssin64, HPC) * np.float32(0.125)
    sk = np.tile(ssin64, HPC)
    rope = np.tile(
        np.concatenate([cq, ck, sq, sk])[None, :], (B, 1)
    ).astype(np.float32)
    eall = np.zeros((B, B * 128), np.float32)
    for b in range(B):
        eall[b, 128 * b : 128 * (b + 1)] = 1.0
    c128 = np.concatenate(
        [np.eye(128, dtype=np.float32), np.ones((128, 1), np.float32)], axis=1
    )

    in_maps = []
    for c in range(NCORES):
        hs = slice(HPC * c, HPC * (c + 1))
        rs = slice(LP * c, LP * (c + 1))
        bqkv = np.tile(
            np.concatenate([bq[rs], bk[rs], bv[rs]])[None, :], (B, 1)
        ).astype(np.float32)
        c8 = np.concatenate([rope, bqkv, eall], axis=1).astype(np.float32)
        def pmajor(a):  # (8, 128, w) -> contiguous (128, 8, w) bf16
            return np.ascontiguousarray(a.transpose(1, 0, 2)).astype(bf16)

        in_maps.append(
            {
                "xt": pmajor(x.T.reshape(8, 128, B)),
                "wq": pmajor(Wq[rs].T.reshape(8, 128, LP)),
                "wk": pmajor(Wk[rs].T.reshape(8, 128, LP)),
                "wv": pmajor(Wv[rs].T.reshape(8, 128, LP)),
                "wo": pmajor(Wo[:, rs].reshape(8, 128, LP).transpose(0, 2, 1)),
                "c8": c8,
                "c128": c128,
                "pk": np.ascontiguousarray(pk_all[:, hs]),
                "pv": np.ascontiguousarray(pv_all[:, hs]),
            }
        )

    nc = _get_nc()
    trace = bool(int(os.environ.get("KERNEL_TRACE", "0")))
    if trace:
        _install_ntff_hook_shim()
    res = run_bass_kernel_spmd(
        nc, in_maps, core_ids=list(range(NCORES)), trace=trace
    )
    kernel.last_results = res

    partial = np.zeros((B, D), np.float32)
    for c in range(NCORES):
        partial = partial + res.results[c]["out"]
    out = partial + bo[None, :]
    return out.reshape(B, S, D).astype(np.float32)


# revision 30
# speedup vs baseline: 1.0500x; 1.0500x over previous
"""Bass/Trainium2 kernel for single-token (decode) self-attention with a
large KV cache, RoPE, and output projection.

Sharding: tensor-parallel over heads. 16 heads / 8 cores = 2 heads per
core; every core sees all 8 batch rows. Per-core HBM traffic is dominated
by its KV-cache slice, which the host downcasts to bf16 (2 x ~16.8MB per
core) — the rel-err budget (2e-2) comfortably absorbs the ~1% noise and
the DMA floor halves vs fp32. QKV weights are sliced by head rows, Wo by
columns (row-parallel out projection); each core returns a partial
(8, 1024) output and the host sums the 8 partials.

Kernel structure per core:
  - q/k/v = x @ W.T + b via PE (fp32), RoPE on DVE with host cos/sin rows
    (q rows also carry the 1/sqrt(hd) attention scale); payload rows are
    broadcast to 128 partitions via one-hot PE matmuls and cast to bf16.
  - K arrives as [128, 64, 64] bf16 with key j = 64*partition + col; V as
    [128, 64, 65] with a host-appended ones column so the softmax
    denominator accumulates in PSUM alongside the context for free.
  - scores: one bf16 tensor_tensor multiply against a 0-stride broadcast
    view of q (DVE 2x perf mode), then a 6-level add-tree over hd
    (levels 1-3 bf16/2x, levels 4-6 fp32 out for precision) — avoids
    tensor_reduce, which is capped at 1x mode.
  - softmax without max subtraction (scores are O(1) by construction);
    exp on ACT producing bf16 probabilities.
  - attn @ V: one PE matmul per 128-key slab, lhsT = prob column
    (LDWEIGHTS overlaps on the second SBUF read port), rhs = [V | 1]
    slab, accumulated into a (1, 16*65) PSUM row holding all 16
    (batch, head) contexts and denominators; the new token contributes
    one extra [1,1]x[1,65] matmul per pair.
  - normalize by 1/denominator, PE-transpose the context row, out-
    projection partial via PE against the host-transposed Wo slice.
"""

import functools
import os
import sys

import numpy as np

for _p in ("/opt/trn_rl_repo", "/root/.axon_site/_ro/trn_rl_repo"):
    if os.path.isdir(_p) and _p not in sys.path:
        sys.path.insert(0, _p)

from contextlib import ExitStack

import concourse.tile as tile
from concourse import bacc, mybir
from concourse.bass_utils import run_bass_kernel_spmd

B, S, D, H, PAST = 8, 1, 1024, 16, 8192
HD = 64
NCORES = 8
HPC = H // NCORES          # heads per core = 2
LP = HPC * HD              # local projection width = 128
NPAIR = B * HPC            # 16 (batch, local-head) problems per core
NCOL = PAST // 128         # 64 keys per partition = score columns per pair
VW = HD + 1                # V slab width incl. denominator ones column
# payload per batch: [q(128) | k(128) | v0,1(65) pad | v1,1(65) pad | e0 _ e1 _]
QBW = 392
PV0 = 2 * LP               # 256: v-block of head 0 (65 wide, ones at +64)
PV1 = 2 * LP + 66          # 322: v-block of head 1
PE0 = 388                  # exp(s_new) head 0; head 1 at 390

F32 = mybir.dt.float32
BF16 = mybir.dt.bfloat16
MULT = mybir.AluOpType.mult
ADD = mybir.AluOpType.add
EXP = mybir.ActivationFunctionType.Exp
ACOPY = mybir.ActivationFunctionType.Copy


def _normalize_pair(nc, ctx_ps, ctxn, dinv, ctxT_ps, ident, p):
    """ctxn[64p:64p+64] = ctx_ps pair block / its denominator; transpose
    the batch row once both of its head pairs are normalized."""
    nc.vector.reciprocal(
        dinv[0:1, p : p + 1], ctx_ps[0:1, VW * p + HD : VW * (p + 1)]
    )
    nc.vector.tensor_scalar_mul(
        ctxn[0:1, HD * p : HD * (p + 1)],
        ctx_ps[0:1, VW * p : VW * p + HD],
        dinv[0:1, p : p + 1],
    )
    if p % HPC == HPC - 1:
        b = p // HPC
        nc.tensor.transpose(
            ctxT_ps[:, b : b + 1],
            ctxn[0:1, 128 * b : 128 * (b + 1)],
            ident[0:1, 0:1],
        )


def _build_bass():
    nc = bacc.Bacc(
        "TRN2", target_bir_lowering=False, debug=False, num_devices=NCORES
    )

    # weights/x arrive partition-major so every DMA line is contiguous
    d_wq = nc.dram_tensor("wq", (128, 8, LP), BF16, kind="ExternalInput").ap()
    d_wk = nc.dram_tensor("wk", (128, 8, LP), BF16, kind="ExternalInput").ap()
    d_wv = nc.dram_tensor("wv", (128, 8, LP), BF16, kind="ExternalInput").ap()
    d_wo = nc.dram_tensor("wo", (128, 8, 128), BF16, kind="ExternalInput").ap()
    d_xt = nc.dram_tensor("xt", (128, 8, B), BF16, kind="ExternalInput").ap()
    # c8: [rope(512) | bqkv(384) | eall(1024)] ; c128: [ident | ones]
    d_c8 = nc.dram_tensor("c8", (B, 1920), F32, kind="ExternalInput").ap()
    d_c128 = nc.dram_tensor("c128", (128, 129), F32, kind="ExternalInput").ap()
    d_pk = nc.dram_tensor(
        "pk", (B, HPC, 128, NCOL, HD), BF16, kind="ExternalInput"
    ).ap()
    d_pv = nc.dram_tensor(
        "pv", (B, HPC, 128, NCOL, VW), BF16, kind="ExternalInput"
    ).ap()
    d_out = nc.dram_tensor("out", (B, D), F32, kind="ExternalOutput").ap()

    with tile.TileContext(nc) as tc:
        with ExitStack() as ctx:
            const = ctx.enter_context(tc.tile_pool(name="const", bufs=1))
            small = ctx.enter_context(tc.tile_pool(name="small", bufs=1))
            wt = ctx.enter_context(tc.tile_pool(name="wt", bufs=1))
            kpool = ctx.enter_context(tc.tile_pool(name="kpool", bufs=6))
            vpool = ctx.enter_context(tc.tile_pool(name="vpool", bufs=7))
            prpool = ctx.enter_context(tc.tile_pool(name="prpool", bufs=2))
            t1pool = ctx.enter_context(tc.tile_pool(name="t1pool", bufs=2))
            t2pool = ctx.enter_context(tc.tile_pool(name="t2pool", bufs=2))
            t3pool = ctx.enter_context(tc.tile_pool(name="t3pool", bufs=3))
            scpool = ctx.enter_context(tc.tile_pool(name="scpool", bufs=3))
            atpool = ctx.enter_context(tc.tile_pool(name="atpool", bufs=4))

            # ---- constants (DMA queue order: small/critical first) --------
            c128 = const.tile([128, 129], F32)
            nc.sync.dma_start(c128[:], d_c128[:])
            c8 = const.tile([B, 1920], F32)
            nc.sync.dma_start(c8[:], d_c8[:])
            ident = c128[:, 0:128]
            rope = c8[:, 0 : 4 * LP]
            bias = c8[:, 4 * LP : 7 * LP]
            eall = c8[:, 7 * LP : 7 * LP + B * 128]

            # ---- prologue: projections, RoPE, bcast --------------------
            with ExitStack() as pctx:
                ps_p = pctx.enter_context(
                    tc.tile_pool(name="ps_p", bufs=1, space="PSUM")
                )
                ps_bc = pctx.enter_context(
                    tc.tile_pool(name="ps_bc", bufs=2, space="PSUM")
                )

                xt = small.tile([128, 8, B], BF16)
                nc.sync.dma_start(xt[:], d_xt[:])
                # Host supplies weights already transposed (in-dim on
                # partitions): wq[p, j, i] = Wq_c[i, 128j+p].
                wts = {}
                for nm, dram in (("q", d_wq), ("k", d_wk), ("v", d_wv)):
                    wtr = wt.tile([128, 8, LP], BF16, tag=f"wt_{nm}")
                    nc.sync.dma_start(wtr[:], dram[:])
                    wts[nm] = wtr

                # payload pads/ones first: no deps, runs during DMA wait
                payload = small.tile([B, QBW], F32)
                nc.vector.memset(payload[:, PV0 + HD : PV0 + HD + 1], 1.0)
                nc.vector.memset(payload[:, PV1 + HD : PV1 + HD + 1], 1.0)
                nc.vector.memset(payload[:, PV1 - 1 : PV1], 0.0)
                nc.vector.memset(payload[:, PV1 + HD + 1 : PE0], 0.0)
                nc.vector.memset(payload[:, PE0 : QBW], 0.0)

                # qkv projection: out (8, 384) = x @ [Wq|Wk|Wv].T
                qkv_ps = ps_p.tile([B, 3 * LP], F32, tag="qkv_ps")
                for i, nm in enumerate(("q", "k", "v")):
                    for j in range(8):
                        nc.tensor.matmul(
                            qkv_ps[:, LP * i : LP * (i + 1)],
                            xt[:, j, :],
                            wts[nm][:, j, :],
                            start=(j == 0),
                            stop=(j == 7),
                        )
                qkv = small.tile([B, 3 * LP], F32)
                nc.vector.tensor_tensor(qkv[:], qkv_ps[:], bias[:], ADD)

                # RoPE on q and k; payload = [rot(q) | rot(k) | v+ones | exp]
                swp = small.tile([B, 2 * LP], F32)  # [q | k] halves swapped
                for i in range(2):  # q, k
                    src = qkv[:, LP * i : LP * (i + 1)].rearrange(
                        "p (h t f) -> p h t f", h=HPC, t=2
                    )
                    dst = swp[:, LP * i : LP * (i + 1)].rearrange(
                        "p (h t f) -> p h t f", h=HPC, t=2
                    )
                    nc.vector.tensor_copy(dst[:, :, 0, :], src[:, :, 1, :])
                    nc.vector.tensor_copy(dst[:, :, 1, :], src[:, :, 0, :])
                tmp = small.tile([B, 2 * LP], F32)
                # tmp = swapped * S ; payload[0:256] = qk * C + tmp
                nc.vector.tensor_tensor(
                    tmp[:], swp[:], rope[:, 2 * LP : 4 * LP], MULT
                )
                nc.vector.tensor_tensor(
                    payload[:, 0 : 2 * LP],
                    qkv[:, 0 : 2 * LP],
                    rope[:, 0 : 2 * LP],
                    MULT,
                )
                nc.vector.tensor_tensor(
                    payload[:, 0 : 2 * LP],
                    payload[:, 0 : 2 * LP],
                    tmp[:],
                    ADD,
                )
                # v blocks (ones columns + pads preset above)
                nc.vector.tensor_copy(
                    payload[:, PV0 : PV0 + HD], qkv[:, 2 * LP : 2 * LP + HD]
                )
                nc.vector.tensor_copy(
                    payload[:, PV1 : PV1 + HD],
                    qkv[:, 2 * LP + HD : 2 * LP + 2 * HD],
                )

                # new-token scores s_new = 0.125 * rot(q).rot(k) per head
                snew = small.tile([B, HPC], F32)
                stt = small.tile([B, HD], F32)
                # q in payload is pre-scaled by 0.125 (folded into rope C/S)
                for hp in range(HPC):
                    nc.vector.scalar_tensor_tensor(
                        out=stt[:],
                        in0=payload[:, LP + HD * hp : LP + HD * (hp + 1)],
                        scalar=1.0,
                        in1=payload[:, HD * hp : HD * (hp + 1)],
                        op0=MULT,
                        op1=MULT,
                        accum_out=snew[:, hp : hp + 1],
                    )
                nc.scalar.activation(
                    payload[:, PE0 : PE0 + 4].rearrange(
                        "p (h t) -> p h t", t=2
                    )[:, :, 0:1],
                    snew[:].rearrange("p (h t) -> p h t", t=1),
                    EXP,
                )

                # broadcast payload rows to all 128 partitions, cast to bf16
                # (per-batch tiles so pair 0 starts before batch 7 lands;
                # casts on ACT so the DVE is free for the first score mult)
                qbs = []
                for b in range(B):
                    bc = ps_bc.tile([128, QBW], F32, tag="bc")
                    nc.tensor.matmul(
                        bc[:],
                        eall[:, 128 * b : 128 * (b + 1)],
                        payload[:],
                        start=True,
                        stop=True,
                    )
                    qb = const.tile([128, QBW], BF16, tag=f"qb{b}")
                    nc.scalar.activation(qb[:], bc[:], ACOPY)
                    qbs.append(qb)

            # ---- main attention loop over the 16 (batch, head) pairs ------
            # ctx_ps row: cols [65p, 65p+64) = context of pair p,
            #             col 65p+64         = softmax denominator of pair p
            ps_ctx = ctx.enter_context(
                tc.tile_pool(name="ps_ctx", bufs=1, space="PSUM")
            )
            ctx_ps = ps_ctx.tile([1, NPAIR * VW], F32)
            ctxT_ps = ps_ctx.tile([128, B], F32, tag="ctxT_ps")
            dinv = small.tile([1, NPAIR], F32)
            ctxn = small.tile([1, NPAIR * HD], F32)

            # Wo tile loads late (issued mid-loop): only the epilogue needs it
            wot = wt.tile([128, 8, 128], BF16, tag="wt_o")

            for p in range(NPAIR):
                b, hp = divmod(p, HPC)
                qb = qbs[b]

                kt = kpool.tile([128, NCOL, HD], BF16, tag="kt")
                nc.sync.dma_start(kt[:], d_pk[b, hp])
                vt = vpool.tile([128, NCOL, VW], BF16, tag="vt")
                nc.sync.dma_start(vt[:], d_pv[b, hp])
                if p == NPAIR - 3:
                    nc.sync.dma_start(wot[:], d_wo[:])

                qslice = qb[:, HD * hp : HD * (hp + 1)]
                qbc = qslice.rearrange("p (o d) -> p o d", o=1).broadcast_to(
                    [128, NCOL, HD]
                )

                # scores: bf16 multiply (2x mode) + 6-level add-tree over hd
                prod = prpool.tile([128, NCOL, HD], BF16, tag="prod")
                nc.vector.tensor_tensor(prod[:], kt[:], qbc, MULT)
                t1 = t1pool.tile([128, NCOL, 32], BF16, tag="t1")
                nc.vector.tensor_tensor(
                    t1[:], prod[:, :, 0:32], prod[:, :, 32:64], ADD
                )
                t2 = t2pool.tile([128, NCOL, 16], BF16, tag="t2")
                nc.vector.tensor_tensor(
                    t2[:], t1[:, :, 0:16], t1[:, :, 16:32], ADD
                )
                t3 = t3pool.tile([128, NCOL, 8], BF16, tag="t3")
                nc.vector.tensor_tensor(
                    t3[:], t2[:, :, 0:8], t2[:, :, 8:16], ADD
                )
                t4 = t1pool.tile([128, NCOL, 4], BF16, tag="t4")
                nc.vector.tensor_tensor(
                    t4[:], t3[:, :, 0:4], t3[:, :, 4:8], ADD
                )
                t5 = t2pool.tile([128, NCOL, 2], BF16, tag="t5")
                nc.vector.tensor_tensor(
                    t5[:], t4[:, :, 0:2], t4[:, :, 2:4], ADD
                )
                sc = scpool.tile([128, NCOL], F32, tag="sc")
                scv = sc[:].rearrange("p (j o) -> p j o", o=1)
                nc.vector.tensor_tensor(
                    scv, t5[:, :, 0:1], t5[:, :, 1:2], ADD
                )

                at = atpool.tile([128, NCOL], BF16, tag="at")
                nc.scalar.activation(at[:], sc[:], EXP)

                # attn @ V (+ denominator via the ones column)
                cslice = ctx_ps[0:1, VW * p : VW * (p + 1)]
                for j in range(NCOL):
                    nc.tensor.matmul(
                        cslice,
                        at[:, j : j + 1],
                        vt[:, j, :],
                        start=(j == 0),
                        stop=False,
                    )
                # new token: exp(s_new) * [v_new | 1]
                pv = PV0 if hp == 0 else PV1
                nc.tensor.matmul(
                    cslice,
                    qb[0:1, PE0 + 2 * hp : PE0 + 2 * hp + 1],
                    qb[0:1, pv : pv + VW],
                    start=False,
                    stop=True,
                )

                # normalize pair p-5 (delayed so the DVE never stalls on the
                # in-flight PE accumulation chain of recent pairs)
                if p >= 5:
                    _normalize_pair(nc, ctx_ps, ctxn, dinv, ctxT_ps, ident, p - 5)

            for pp in range(NPAIR - 5, NPAIR):
                _normalize_pair(nc, ctx_ps, ctxn, dinv, ctxT_ps, ident, pp)

            # ---- finalize: out-projection --------------------------------
            ctxT = small.tile([128, B], BF16)
            nc.vector.tensor_copy(ctxT[:], ctxT_ps[:])

            outsb = small.tile([B, D], F32)
            for half in range(2):
                op_ps = ps_ctx.tile([B, 512], F32, tag="op_ps")
                nc.tensor.matmul(
                    op_ps[:],
                    ctxT[:],
                    wot[:, 4 * half : 4 * (half + 1), :],
                    start=True,
                    stop=True,
                )
                nc.vector.tensor_copy(
                    outsb[:, 512 * half : 512 * (half + 1)], op_ps[:]
                )
            nc.sync.dma_start(d_out[:], outsb[:])

    nc.compile()
    return nc


@functools.lru_cache(maxsize=1)
def _get_nc():
    return _build_bass()


def _rope_tables():
    """cos/sin rows for position PAST, mirroring reference.py's fp32 jax
    arithmetic so the tables round identically."""
    import jax
    import jax.numpy as jnp

    pos = (PAST + jnp.arange(S)).astype(jnp.float32)
    inv_freq = 1.0 / (
        10000.0 ** (jnp.arange(0, HD, 2, dtype=jnp.float32) / HD)
    )
    ang = pos[:, None] * inv_freq[None, :]
    cos32 = np.asarray(jnp.cos(ang))[0]
    sin32 = np.asarray(jnp.sin(ang))[0]
    cos64 = np.concatenate([cos32, cos32])
    ssin64 = np.concatenate([-sin32, sin32])
    return cos64.astype(np.float32), ssin64.astype(np.float32)


def _install_ntff_hook_shim():
    """The agent image's antenv stub lacks axon_hooks, which degrades
    run_bass_kernel_spmd(trace=True) into an ImportError. Provide the
    module and register the ctypes-based NTFF hook from trn_agent_boot."""
    import types

    try:
        import antenv.axon_hooks  # noqa: F401

        return
    except ImportError:
        pass
    try:
        import antenv
        from trn_agent_boot.trn_boot import _ntff_profile_via_ctypes

        mod = types.ModuleType("antenv.axon_hooks")
        _state = {"hook": _ntff_profile_via_ctypes("/opt/axon/libaxon_pjrt.so")}
        mod.get_axon_ntff_profile_hook = lambda: _state["hook"]
        mod.set_axon_ntff_profile_hook = lambda h: _state.update(hook=h)
        sys.modules["antenv.axon_hooks"] = mod
        antenv.axon_hooks = mod
    except Exception as e:  # profiling is best-effort
        print(f"ntff hook shim failed: {e}", file=sys.stderr)


@functools.lru_cache(maxsize=1)
def _bf16():
    import ml_dtypes

    return np.dtype(ml_dtypes.bfloat16)


def kernel(x, Wq, bq, Wk, bk, Wv, bv, Wo, bo, past_k, past_v):
    x = np.asarray(x, np.float32).reshape(B, D)
    Wq = np.asarray(Wq, np.float32)
    Wk = np.asarray(Wk, np.float32)
    Wv = np.asarray(Wv, np.float32)
    Wo = np.asarray(Wo, np.float32)
    bq = np.asarray(bq, np.float32)
    bk = np.asarray(bk, np.float32)
    bv = np.asarray(bv, np.float32)
    bo = np.asarray(bo, np.float32)

    bf16 = _bf16()
    # K: (B, H, 128, 64, 64) bf16 with key = 64*partition + col
    pk_all = np.ascontiguousarray(past_k, np.float32).reshape(
        B, H, 128, NCOL, HD
    ).astype(bf16)
    # V: (B, H, 128, 64, 65) bf16 with a ones column appended per slab
    pv_all = np.empty((B, H, 128, NCOL, VW), dtype=bf16)
    pv_all[..., :HD] = np.ascontiguousarray(past_v, np.float32).reshape(
        B, H, 128, NCOL, HD
    ).astype(bf16)
    pv_all[..., HD] = np.float32(1.0)

    cos64, ssin64 = _rope_tables()
    # C/S for the q columns carry the 1/sqrt(hd) attention scale
    cq = np.tile(cos64, HPC) * np.float32(0.125)
    ck = np.tile(cos64, HPC)
    sq = np.tile(ssin64, HPC) * np.float32(0.125)
    sk = np.tile(ssin64, HPC)
    rope = np.tile(
        np.concatenate([cq, ck, sq, sk])[None, :], (B, 1)
    ).astype(np.float32)
    eall = np.zeros((B, B * 128), np.float32)
    for b in range(B):
        eall[b, 128 * b : 128 * (b + 1)] = 1.0
    c128 = np.concatenate(
        [np.eye(128, dtype=np.float32), np.ones((128, 1), np.float32)], axis=1
    )

    in_maps = []
    for c in range(NCORES):
        hs = slice(HPC * c, HPC * (c + 1))
        rs = slice(LP * c, LP * (c + 1))
        bqkv = np.tile(
            np.concatenate([bq[rs], bk[rs], bv[rs]])[None, :], (B, 1)
        ).astype(np.float32)
        c8 = np.concatenate([rope, bqkv, eall], axis=1).astype(np.float32)
        def pmajor(a):  # (8, 128, w) -> contiguous (128, 8, w) bf16
            return np.ascontiguousarray(a.transpose(1, 0, 2)).astype(bf16)

        in_maps.append(
            {
                "xt": pmajor(x.T.reshape(8, 128, B)),
                "wq": pmajor(Wq[rs].T.reshape(8, 128, LP)),
                "wk": pmajor(Wk[rs].T.reshape(8, 128, LP)),
                "wv": pmajor(Wv[rs].T.reshape(8, 128, LP)),
                "wo": pmajor(Wo[:, rs].reshape(8, 128, LP).transpose(0, 2, 1)),
                "c8": c8,
                "c128": c128,
                "pk": np.ascontiguousarray(pk_all[:, hs]),
                "pv": np.ascontiguousarray(pv_all[:, hs]),
            }
        )

    nc = _get_nc()
    trace = bool(int(os.environ.get("KERNEL_TRACE", "0")))
    if trace:
        _install_ntff_hook_shim()
    res = run_bass_kernel_spmd(
        nc, in_maps, core_ids=list(range(NCORES)), trace=trace
    )
    kernel.last_results = res

    partial = np.zeros((B, D), np.float32)
    for c in range(NCORES):
        partial = partial + res.results[c]["out"]
    out = partial + bo[None, :]
    return out.reshape(B, S, D).astype(np.float32)


# revision 31
# speedup vs baseline: 1.0641x; 1.0134x over previous
"""Bass/Trainium2 kernel for single-token (decode) self-attention with a
large KV cache, RoPE, and output projection.

Sharding: tensor-parallel over heads. 16 heads / 8 cores = 2 heads per
core; every core sees all 8 batch rows. Per-core HBM traffic is dominated
by its KV-cache slice, which the host downcasts to bf16 (2 x ~16.8MB per
core) — the rel-err budget (2e-2) comfortably absorbs the ~1% noise and
the DMA floor halves vs fp32. QKV weights are sliced by head rows, Wo by
columns (row-parallel out projection); each core returns a partial
(8, 1024) output and the host sums the 8 partials.

Kernel structure per core:
  - q/k/v = x @ W.T + b via PE (fp32), RoPE on DVE with host cos/sin rows
    (q rows also carry the 1/sqrt(hd) attention scale); payload rows are
    broadcast to 128 partitions via one-hot PE matmuls and cast to bf16.
  - K arrives as [128, 64, 64] bf16 with key j = 64*partition + col; V as
    [128, 64, 65] with a host-appended ones column so the softmax
    denominator accumulates in PSUM alongside the context for free.
  - scores: one bf16 tensor_tensor multiply against a 0-stride broadcast
    view of q (DVE 2x perf mode), then a 6-level add-tree over hd
    (levels 1-3 bf16/2x, levels 4-6 fp32 out for precision) — avoids
    tensor_reduce, which is capped at 1x mode.
  - softmax without max subtraction (scores are O(1) by construction);
    exp on ACT producing bf16 probabilities.
  - attn @ V: one PE matmul per 128-key slab, lhsT = prob column
    (LDWEIGHTS overlaps on the second SBUF read port), rhs = [V | 1]
    slab, accumulated into a (1, 16*65) PSUM row holding all 16
    (batch, head) contexts and denominators; the new token contributes
    one extra [1,1]x[1,65] matmul per pair.
  - normalize by 1/denominator, PE-transpose the context row, out-
    projection partial via PE against the host-transposed Wo slice.
"""

import functools
import os
import sys

import numpy as np

for _p in ("/opt/trn_rl_repo", "/root/.axon_site/_ro/trn_rl_repo"):
    if os.path.isdir(_p) and _p not in sys.path:
        sys.path.insert(0, _p)

from contextlib import ExitStack

import concourse.tile as tile
from concourse import bacc, mybir
from concourse.bass_utils import run_bass_kernel_spmd

B, S, D, H, PAST = 8, 1, 1024, 16, 8192
HD = 64
NCORES = 8
HPC = H // NCORES          # heads per core = 2
LP = HPC * HD              # local projection width = 128
NPAIR = B * HPC            # 16 (batch, local-head) problems per core
NCOL = PAST // 128         # 64 keys per partition = score columns per pair
VW = HD + 1                # V slab width incl. denominator ones column
# payload per batch: [q(128) | k(128) | v0,1(65) pad | v1,1(65) pad | e0 _ e1 _]
QBW = 392
PV0 = 2 * LP               # 256: v-block of head 0 (65 wide, ones at +64)
PV1 = 2 * LP + 66          # 322: v-block of head 1
PE0 = 388                  # exp(s_new) head 0; head 1 at 390

F32 = mybir.dt.float32
BF16 = mybir.dt.bfloat16
MULT = mybir.AluOpType.mult
ADD = mybir.AluOpType.add
EXP = mybir.ActivationFunctionType.Exp
ACOPY = mybir.ActivationFunctionType.Copy


def _normalize_pair(nc, ctx_ps, ctxn, dinv, ctxT_ps, ident, p):
    """ctxn[64p:64p+64] = ctx_ps pair block / its denominator; transpose
    the batch row once both of its head pairs are normalized."""
    nc.vector.reciprocal(
        dinv[0:1, p : p + 1], ctx_ps[0:1, VW * p + HD : VW * (p + 1)]
    )
    nc.vector.tensor_scalar_mul(
        ctxn[0:1, HD * p : HD * (p + 1)],
        ctx_ps[0:1, VW * p : VW * p + HD],
        dinv[0:1, p : p + 1],
    )
    if p % HPC == HPC - 1:
        b = p // HPC
        nc.tensor.transpose(
            ctxT_ps[:, b : b + 1],
            ctxn[0:1, 128 * b : 128 * (b + 1)],
            ident[0:1, 0:1],
        )


def _build_bass():
    nc = bacc.Bacc(
        "TRN2", target_bir_lowering=False, debug=False, num_devices=NCORES
    )

    # weights/x arrive partition-major so every DMA line is contiguous
    d_wq = nc.dram_tensor("wq", (128, 8, LP), BF16, kind="ExternalInput").ap()
    d_wk = nc.dram_tensor("wk", (128, 8, LP), BF16, kind="ExternalInput").ap()
    d_wv = nc.dram_tensor("wv", (128, 8, LP), BF16, kind="ExternalInput").ap()
    d_wo = nc.dram_tensor("wo", (128, 8, 128), BF16, kind="ExternalInput").ap()
    d_xt = nc.dram_tensor("xt", (128, 8, B), BF16, kind="ExternalInput").ap()
    # c8: [rope(512) | bqkv(384) | eall(1024)] ; c128: [ident | ones]
    d_c8 = nc.dram_tensor("c8", (B, 1920), F32, kind="ExternalInput").ap()
    d_c128 = nc.dram_tensor("c128", (128, 129), F32, kind="ExternalInput").ap()
    d_pk = nc.dram_tensor(
        "pk", (B, HPC, 128, NCOL, HD), BF16, kind="ExternalInput"
    ).ap()
    d_pv = nc.dram_tensor(
        "pv", (B, HPC, 128, NCOL, VW), BF16, kind="ExternalInput"
    ).ap()
    d_out = nc.dram_tensor("out", (B, D), F32, kind="ExternalOutput").ap()

    with tile.TileContext(nc) as tc:
        with ExitStack() as ctx:
            const = ctx.enter_context(tc.tile_pool(name="const", bufs=1))
            small = ctx.enter_context(tc.tile_pool(name="small", bufs=1))
            wt = ctx.enter_context(tc.tile_pool(name="wt", bufs=1))
            kpool = ctx.enter_context(tc.tile_pool(name="kpool", bufs=6))
            vpool = ctx.enter_context(tc.tile_pool(name="vpool", bufs=7))
            prpool = ctx.enter_context(tc.tile_pool(name="prpool", bufs=2))
            t1pool = ctx.enter_context(tc.tile_pool(name="t1pool", bufs=2))
            t2pool = ctx.enter_context(tc.tile_pool(name="t2pool", bufs=2))
            t3pool = ctx.enter_context(tc.tile_pool(name="t3pool", bufs=3))
            scpool = ctx.enter_context(tc.tile_pool(name="scpool", bufs=3))
            atpool = ctx.enter_context(tc.tile_pool(name="atpool", bufs=4))

            # ---- constants (DMA queue order: small/critical first) --------
            c128 = const.tile([128, 129], F32)
            nc.scalar.dma_start(c128[:], d_c128[:])
            c8 = const.tile([B, 1920], F32)
            nc.scalar.dma_start(c8[:], d_c8[:])
            ident = c128[:, 0:128]
            rope = c8[:, 0 : 4 * LP]
            bias = c8[:, 4 * LP : 7 * LP]
            eall = c8[:, 7 * LP : 7 * LP + B * 128]

            # ---- prologue: projections, RoPE, bcast --------------------
            with ExitStack() as pctx:
                ps_p = pctx.enter_context(
                    tc.tile_pool(name="ps_p", bufs=1, space="PSUM")
                )
                ps_bc = pctx.enter_context(
                    tc.tile_pool(name="ps_bc", bufs=2, space="PSUM")
                )

                xt = small.tile([128, 8, B], BF16)
                nc.scalar.dma_start(xt[:], d_xt[:])
                # Host supplies weights already transposed (in-dim on
                # partitions): wq[p, j, i] = Wq_c[i, 128j+p].
                wts = {}
                for nm, dram in (("q", d_wq), ("k", d_wk), ("v", d_wv)):
                    wtr = wt.tile([128, 8, LP], BF16, tag=f"wt_{nm}")
                    nc.scalar.dma_start(wtr[:], dram[:])
                    wts[nm] = wtr

                # payload pads/ones first: no deps, runs during DMA wait
                payload = small.tile([B, QBW], F32)
                nc.vector.memset(payload[:, PV0 + HD : PV0 + HD + 1], 1.0)
                nc.vector.memset(payload[:, PV1 + HD : PV1 + HD + 1], 1.0)
                nc.vector.memset(payload[:, PV1 - 1 : PV1], 0.0)
                nc.vector.memset(payload[:, PV1 + HD + 1 : PE0], 0.0)
                nc.vector.memset(payload[:, PE0 : QBW], 0.0)

                # qkv projection: out (8, 384) = x @ [Wq|Wk|Wv].T
                qkv_ps = ps_p.tile([B, 3 * LP], F32, tag="qkv_ps")
                for i, nm in enumerate(("q", "k", "v")):
                    for j in range(8):
                        nc.tensor.matmul(
                            qkv_ps[:, LP * i : LP * (i + 1)],
                            xt[:, j, :],
                            wts[nm][:, j, :],
                            start=(j == 0),
                            stop=(j == 7),
                        )
                qkv = small.tile([B, 3 * LP], F32)
                nc.vector.tensor_tensor(qkv[:], qkv_ps[:], bias[:], ADD)

                # RoPE on q and k; payload = [rot(q) | rot(k) | v+ones | exp]
                swp = small.tile([B, 2 * LP], F32)  # [q | k] halves swapped
                for i in range(2):  # q, k
                    src = qkv[:, LP * i : LP * (i + 1)].rearrange(
                        "p (h t f) -> p h t f", h=HPC, t=2
                    )
                    dst = swp[:, LP * i : LP * (i + 1)].rearrange(
                        "p (h t f) -> p h t f", h=HPC, t=2
                    )
                    nc.vector.tensor_copy(dst[:, :, 0, :], src[:, :, 1, :])
                    nc.vector.tensor_copy(dst[:, :, 1, :], src[:, :, 0, :])
                tmp = small.tile([B, 2 * LP], F32)
                # tmp = swapped * S ; payload[0:256] = qk * C + tmp
                nc.vector.tensor_tensor(
                    tmp[:], swp[:], rope[:, 2 * LP : 4 * LP], MULT
                )
                nc.vector.tensor_tensor(
                    payload[:, 0 : 2 * LP],
                    qkv[:, 0 : 2 * LP],
                    rope[:, 0 : 2 * LP],
                    MULT,
                )
                nc.vector.tensor_tensor(
                    payload[:, 0 : 2 * LP],
                    payload[:, 0 : 2 * LP],
                    tmp[:],
                    ADD,
                )
                # v blocks (ones columns + pads preset above)
                nc.vector.tensor_copy(
                    payload[:, PV0 : PV0 + HD], qkv[:, 2 * LP : 2 * LP + HD]
                )
                nc.vector.tensor_copy(
                    payload[:, PV1 : PV1 + HD],
                    qkv[:, 2 * LP + HD : 2 * LP + 2 * HD],
                )

                # new-token scores s_new = 0.125 * rot(q).rot(k) per head
                snew = small.tile([B, HPC], F32)
                stt = small.tile([B, HD], F32)
                # q in payload is pre-scaled by 0.125 (folded into rope C/S)
                for hp in range(HPC):
                    nc.vector.scalar_tensor_tensor(
                        out=stt[:],
                        in0=payload[:, LP + HD * hp : LP + HD * (hp + 1)],
                        scalar=1.0,
                        in1=payload[:, HD * hp : HD * (hp + 1)],
                        op0=MULT,
                        op1=MULT,
                        accum_out=snew[:, hp : hp + 1],
                    )
                nc.scalar.activation(
                    payload[:, PE0 : PE0 + 4].rearrange(
                        "p (h t) -> p h t", t=2
                    )[:, :, 0:1],
                    snew[:].rearrange("p (h t) -> p h t", t=1),
                    EXP,
                )

                # broadcast payload rows to all 128 partitions, cast to bf16
                # (per-batch tiles so pair 0 starts before batch 7 lands;
                # casts on ACT so the DVE is free for the first score mult)
                qbs = []
                for b in range(B):
                    bc = ps_bc.tile([128, QBW], F32, tag="bc")
                    nc.tensor.matmul(
                        bc[:],
                        eall[:, 128 * b : 128 * (b + 1)],
                        payload[:],
                        start=True,
                        stop=True,
                    )
                    qb = const.tile([128, QBW], BF16, tag=f"qb{b}")
                    nc.scalar.activation(qb[:], bc[:], ACOPY)
                    qbs.append(qb)

            # ---- main attention loop over the 16 (batch, head) pairs ------
            # ctx_ps row: cols [65p, 65p+64) = context of pair p,
            #             col 65p+64         = softmax denominator of pair p
            ps_ctx = ctx.enter_context(
                tc.tile_pool(name="ps_ctx", bufs=1, space="PSUM")
            )
            ctx_ps = ps_ctx.tile([1, NPAIR * VW], F32)
            ctxT_ps = ps_ctx.tile([128, B], F32, tag="ctxT_ps")
            dinv = small.tile([1, NPAIR], F32)
            ctxn = small.tile([1, NPAIR * HD], F32)

            # Wo tile loads late (issued mid-loop): only the epilogue needs it
            wot = wt.tile([128, 8, 128], BF16, tag="wt_o")

            for p in range(NPAIR):
                b, hp = divmod(p, HPC)
                qb = qbs[b]

                kt = kpool.tile([128, NCOL, HD], BF16, tag="kt")
                nc.sync.dma_start(kt[:], d_pk[b, hp])
                vt = vpool.tile([128, NCOL, VW], BF16, tag="vt")
                nc.scalar.dma_start(vt[:], d_pv[b, hp])
                if p == NPAIR - 3:
                    nc.scalar.dma_start(wot[:], d_wo[:])

                qslice = qb[:, HD * hp : HD * (hp + 1)]
                qbc = qslice.rearrange("p (o d) -> p o d", o=1).broadcast_to(
                    [128, NCOL, HD]
                )

                # scores: bf16 multiply (2x mode) + 6-level add-tree over hd
                prod = prpool.tile([128, NCOL, HD], BF16, tag="prod")
                nc.vector.tensor_tensor(prod[:], kt[:], qbc, MULT)
                t1 = t1pool.tile([128, NCOL, 32], BF16, tag="t1")
                nc.vector.tensor_tensor(
                    t1[:], prod[:, :, 0:32], prod[:, :, 32:64], ADD
                )
                t2 = t2pool.tile([128, NCOL, 16], BF16, tag="t2")
                nc.vector.tensor_tensor(
                    t2[:], t1[:, :, 0:16], t1[:, :, 16:32], ADD
                )
                t3 = t3pool.tile([128, NCOL, 8], BF16, tag="t3")
                nc.vector.tensor_tensor(
                    t3[:], t2[:, :, 0:8], t2[:, :, 8:16], ADD
                )
                t4 = t1pool.tile([128, NCOL, 4], BF16, tag="t4")
                nc.vector.tensor_tensor(
                    t4[:], t3[:, :, 0:4], t3[:, :, 4:8], ADD
                )
                t5 = t2pool.tile([128, NCOL, 2], BF16, tag="t5")
                nc.vector.tensor_tensor(
                    t5[:], t4[:, :, 0:2], t4[:, :, 2:4], ADD
                )
                sc = scpool.tile([128, NCOL], F32, tag="sc")
                scv = sc[:].rearrange("p (j o) -> p j o", o=1)
                nc.vector.tensor_tensor(
                    scv, t5[:, :, 0:1], t5[:, :, 1:2], ADD
                )

                at = atpool.tile([128, NCOL], BF16, tag="at")
                nc.scalar.activation(at[:], sc[:], EXP)

                # attn @ V (+ denominator via the ones column)
                cslice = ctx_ps[0:1, VW * p : VW * (p + 1)]
                for j in range(NCOL):
                    nc.tensor.matmul(
                        cslice,
                        at[:, j : j + 1],
                        vt[:, j, :],
                        start=(j == 0),
                        stop=False,
                    )
                # new token: exp(s_new) * [v_new | 1]
                pv = PV0 if hp == 0 else PV1
                nc.tensor.matmul(
                    cslice,
                    qb[0:1, PE0 + 2 * hp : PE0 + 2 * hp + 1],
                    qb[0:1, pv : pv + VW],
                    start=False,
                    stop=True,
                )

                # normalize pair p-5 (delayed so the DVE never stalls on the
                # in-flight PE accumulation chain of recent pairs)
                if p >= 5:
                    _normalize_pair(nc, ctx_ps, ctxn, dinv, ctxT_ps, ident, p - 5)

            for pp in range(NPAIR - 5, NPAIR):
                _normalize_pair(nc, ctx_ps, ctxn, dinv, ctxT_ps, ident, pp)

            # ---- finalize: out-projection --------------------------------
            ctxT = small.tile([128, B], BF16)
            nc.vector.tensor_copy(ctxT[:], ctxT_ps[:])

            outsb = small.tile([B, D], F32)
            for half in range(2):
                op_ps = ps_ctx.tile([B, 512], F32, tag="op_ps")
                nc.tensor.matmul(
                    op_ps[:],
                    ctxT[:],
                    wot[:, 4 * half : 4 * (half + 1), :],
                    start=True,
                    stop=True,
                )
                nc.vector.tensor_copy(
                    outsb[:, 512 * half : 512 * (half + 1)], op_ps[:]
                )
            nc.sync.dma_start(d_out[:], outsb[:])

    nc.compile()
    return nc


@functools.lru_cache(maxsize=1)
def _get_nc():
    return _build_bass()


def _rope_tables():
    """cos/sin rows for position PAST, mirroring reference.py's fp32 jax
    arithmetic so the tables round identically."""
    import jax
    import jax.numpy as jnp

    pos = (PAST + jnp.arange(S)).astype(jnp.float32)
    inv_freq = 1.0 / (
        10000.0 ** (jnp.arange(0, HD, 2, dtype=jnp.float32) / HD)
    )
    ang = pos[:, None] * inv_freq[None, :]
    cos32 = np.asarray(jnp.cos(ang))[0]
    sin32 = np.asarray(jnp.sin(ang))[0]
    cos64 = np.concatenate([cos32, cos32])
    ssin64 = np.concatenate([-sin32, sin32])
    return cos64.astype(np.float32), ssin64.astype(np.float32)


def _install_ntff_hook_shim():
    """The agent image's antenv stub lacks axon_hooks, which degrades
    run_bass_kernel_spmd(trace=True) into an ImportError. Provide the
    module and register the ctypes-based NTFF hook from trn_agent_boot."""
    import types

    try:
        import antenv.axon_hooks  # noqa: F401

        return
    except ImportError:
        pass
    try:
        import antenv
        from trn_agent_boot.trn_boot import _ntff_profile_via_ctypes

        mod = types.ModuleType("antenv.axon_hooks")
        _state = {"hook": _ntff_profile_via_ctypes("/opt/axon/libaxon_pjrt.so")}
        mod.get_axon_ntff_profile_hook = lambda: _state["hook"]
        mod.set_axon_ntff_profile_hook = lambda h: _state.update(hook=h)
        sys.modules["antenv.axon_hooks"] = mod
        antenv.axon_hooks = mod
    except Exception as e:  # profiling is best-effort
        print(f"ntff hook shim failed: {e}", file=sys.stderr)


@functools.lru_cache(maxsize=1)
def _bf16():
    import ml_dtypes

    return np.dtype(ml_dtypes.bfloat16)


def kernel(x, Wq, bq, Wk, bk, Wv, bv, Wo, bo, past_k, past_v):
    x = np.asarray(x, np.float32).reshape(B, D)
    Wq = np.asarray(Wq, np.float32)
    Wk = np.asarray(Wk, np.float32)
    Wv = np.asarray(Wv, np.float32)
    Wo = np.asarray(Wo, np.float32)
    bq = np.asarray(bq, np.float32)
    bk = np.asarray(bk, np.float32)
    bv = np.asarray(bv, np.float32)
    bo = np.asarray(bo, np.float32)

    bf16 = _bf16()
    # K: (B, H, 128, 64, 64) bf16 with key = 64*partition + col
    pk_all = np.ascontiguousarray(past_k, np.float32).reshape(
        B, H, 128, NCOL, HD
    ).astype(bf16)
    # V: (B, H, 128, 64, 65) bf16 with a ones column appended per slab
    pv_all = np.empty((B, H, 128, NCOL, VW), dtype=bf16)
    pv_all[..., :HD] = np.ascontiguousarray(past_v, np.float32).reshape(
        B, H, 128, NCOL, HD
    ).astype(bf16)
    pv_all[..., HD] = np.float32(1.0)

    cos64, ssin64 = _rope_tables()
    # C/S for the q columns carry the 1/sqrt(hd) attention scale
    cq = np.tile(cos64, HPC) * np.float32(0.125)
    ck = np.tile(cos64, HPC)
    sq = np.tile(ssin64, HPC) * np.float32(0.125)
    sk = np.tile(ssin64, HPC)
    rope = np.tile(
        np.concatenate([cq, ck, sq, sk])[None, :], (B, 1)
    ).astype(np.float32)
    eall = np.zeros((B, B * 128), np.float32)
    for b in range(B):
        eall[b, 128 * b : 128 * (b + 1)] = 1.0
    c128 = np.concatenate(
        [np.eye(128, dtype=np.float32), np.ones((128, 1), np.float32)], axis=1
    )

    in_maps = []
    for c in range(NCORES):
        hs = slice(HPC * c, HPC * (c + 1))
        rs = slice(LP * c, LP * (c + 1))
        bqkv = np.tile(
            np.concatenate([bq[rs], bk[rs], bv[rs]])[None, :], (B, 1)
        ).astype(np.float32)
        c8 = np.concatenate([rope, bqkv, eall], axis=1).astype(np.float32)
        def pmajor(a):  # (8, 128, w) -> contiguous (128, 8, w) bf16
            return np.ascontiguousarray(a.transpose(1, 0, 2)).astype(bf16)

        in_maps.append(
            {
                "xt": pmajor(x.T.reshape(8, 128, B)),
                "wq": pmajor(Wq[rs].T.reshape(8, 128, LP)),
                "wk": pmajor(Wk[rs].T.reshape(8, 128, LP)),
                "wv": pmajor(Wv[rs].T.reshape(8, 128, LP)),
                "wo": pmajor(Wo[:, rs].reshape(8, 128, LP).transpose(0, 2, 1)),
                "c8": c8,
                "c128": c128,
                "pk": np.ascontiguousarray(pk_all[:, hs]),
                "pv": np.ascontiguousarray(pv_all[:, hs]),
            }
        )

    nc = _get_nc()
    trace = bool(int(os.environ.get("KERNEL_TRACE", "0")))
    if trace:
        _install_ntff_hook_shim()
    res = run_bass_kernel_spmd(
        nc, in_maps, core_ids=list(range(NCORES)), trace=trace
    )
    kernel.last_results = res

    partial = np.zeros((B, D), np.float32)
    for c in range(NCORES):
        partial = partial + res.results[c]["out"]
    out = partial + bo[None, :]
    return out.reshape(B, S, D).astype(np.float32)
